# revision 14
# baseline (speedup 1.0000x reference)
"""EnhancedGovernanceAttention Trainium2 kernel (8 NeuronCores, SPMD).

Sharding: core c owns heads {2c, 2c+1} for BOTH batches. Each core computes
its heads' attention and a row-parallel partial of the Wo projection; the
host sums the 8 partials and adds bo.

Math notes (vs the jax reference):
 - softmax max-subtraction is dropped: scores ~ N(0,1) + bias in [0,0.3],
   so exp() cannot overflow; softmax is shift-invariant.
 - log1p memory bias log(1 + GS*mw + 1e-8) = log(w) is per-(batch,key):
   it rides the exp's per-partition bias operand (exact f32), so
   exp(score+logw) carries w for both the PV numerator and the row-sum.
 - the remaining bias (GS*policy + causal -40 mask) is batch-INDEPENDENT:
   phase B runs batch-inner so each fp8 slab is DMA'd once and used by
   both batches' m-loops.
 - causal mask: only lower-triangle k-tiles are computed; the intra-tile
   diagonal mask is baked into the (fp8) bias as -40.
 - scores are computed TRANSPOSED ([k, q]) so the PV matmul directly
   yields attn outputs o^T = (PV)^T/l, which is the lhsT the output
   projection needs.
 - QKV projections run as fp8 DoubleRow matmuls on a hi/lo split of x and
   64*W (3 cross terms; the 64x pre-scale keeps the lo residuals out of
   fp8's subnormal range; 1/64 is folded into the RoPE tables and the V
   staging copy). V is computed transposed like q/k (wide chains, ~4x
   fewer PE instructions) and flipped to [s, hd] with PE transposes.
 - the policy bias is added into the score PSUM with an fp8 DoubleRow
   identity matmul (2 k-tiles per slab pair, [I;0]/[0;I] selects the slot).
 - softmax denominator: exp tiles accumulate into U on the DVE; one Pool
   partition_all_reduce per (b,h,q-block) yields the row sums broadcast
   to all partitions (no ones-matmul, no separate broadcast).
 - output projection in fp8 DoubleRow: the tail splits o^T (x64) into
   fp8 hi+lo; C-units contract BOTH heads (256 ch) per instruction with
   3 cross terms against 64*Wo hi/lo; the y staging copy descales 1/4096.
 - y is written as contiguous [B, ST, 128, D] tiles (one descriptor run
   per DMA, 4 C-units batched per DMA); host reassembles.
"""

import numpy as np
import ml_dtypes
from contextlib import ExitStack

import concourse.bass as bass
import concourse.tile as tile
from concourse import bacc, bass_isa, mybir
from concourse.bass_utils import run_bass_kernel_spmd
from concourse.masks import make_identity

B, S, D, H, HD = 2, 2048, 2048, 16, 128
GS = 0.1
ROPE_BASE = 10000.0
NCORES = 8
HPC = H // NCORES          # heads per core = 2
SCALE = float(HD) ** -0.5
DT = D // 128              # 16 d-tiles
GT = DT // 2               # 8 d-tile pairs (DoubleRow)
ST = S // 128              # 16 s-tiles (also k-tiles)
QB = 512                   # q-block width (phase B)
NQB = S // QB              # 4 q-blocks
SB = 512                   # s-block width (phase A panels)
NSB = S // SB              # 4 s-blocks
MASK_NEG = -40.0
WSC = 64.0                 # fp8 weight pre-scale
OSC = 64.0                 # fp8 attn-output pre-scale

F32 = mybir.dt.float32
BF16 = mybir.dt.bfloat16
FP8 = mybir.dt.float8e4
EXP = mybir.ActivationFunctionType.Exp
CPY = mybir.ActivationFunctionType.Copy
DR = mybir.MatmulPerfMode.DoubleRow
MUL = mybir.AluOpType.mult
SUB = mybir.AluOpType.subtract
RADD = bass_isa.ReduceOp.add

_CACHE = {}


def build_nc():
    nc = bacc.Bacc("TRN2", target_bir_lowering=False, debug=False,
                   num_devices=NCORES)

    d_xhi = nc.dram_tensor("xhi", [B, 128, GT, 2, S], FP8, kind="ExternalInput").ap()
    d_xlo = nc.dram_tensor("xlo", [B, 128, GT, 2, S], FP8, kind="ExternalInput").ap()
    CC = HPC * HD
    d_w = {}
    for nm in ("qhi", "qlo", "khi", "klo", "vhi", "vlo"):
        d_w[nm] = nc.dram_tensor(f"w{nm}", [128, GT, 2, CC], FP8,
                                 kind="ExternalInput").ap()
    d_woh = nc.dram_tensor("woh", [128, HPC, D], FP8, kind="ExternalInput").ap()
    d_wol = nc.dram_tensor("wol", [128, HPC, D], FP8, kind="ExternalInput").ap()
    d_id8 = nc.dram_tensor("id8", [128, 3, 128], FP8, kind="ExternalInput").ap()
    d_bias = nc.dram_tensor("biasT", [HPC, S, S], FP8, kind="ExternalInput").ap()
    d_cs = nc.dram_tensor("cs", [128, 2, S], BF16, kind="ExternalInput").ap()
    d_logw = nc.dram_tensor("logw", [128, B, ST], F32, kind="ExternalInput").ap()
    d_y = nc.dram_tensor("y", [B, ST, 128, D], BF16, kind="ExternalOutput").ap()

    with tile.TileContext(nc) as tc, ExitStack() as ctx:
        consts = ctx.enter_context(tc.tile_pool(name="consts", bufs=1))
        wpool = ctx.enter_context(tc.tile_pool(name="wpool", bufs=1))
        qkv = ctx.enter_context(tc.tile_pool(name="qkv", bufs=1))
        panels = ctx.enter_context(tc.tile_pool(name="panels", bufs=2))
        rope = ctx.enter_context(tc.tile_pool(name="rope", bufs=3))
        vtsp = ctx.enter_context(tc.tile_pool(name="vtsp", bufs=2))
        slabs = ctx.enter_context(tc.tile_pool(name="slabs", bufs=2))
        expp = ctx.enter_context(tc.tile_pool(name="expp", bufs=4))
        upool = ctx.enter_context(tc.tile_pool(name="upool", bufs=3))
        normp = ctx.enter_context(tc.tile_pool(name="normp", bufs=2))
        outp = ctx.enter_context(tc.tile_pool(name="outp", bufs=4))
        psum_mm = ctx.enter_context(tc.tile_pool(name="psum_mm", bufs=3, space="PSUM"))
        psum_pv = ctx.enter_context(tc.tile_pool(name="psum_pv", bufs=2, space="PSUM"))
        psum_c = ctx.enter_context(tc.tile_pool(name="psum_c", bufs=2, space="PSUM"))
        psum_l = ctx.enter_context(tc.tile_pool(name="psum_l", bufs=1, space="PSUM"))

        # ---------------- constants (emission order = queue priority) ----------
        # ACT queue: q/k weights first (first chains need them); halves so the
        # cold-start chains can begin on the first half.
        t_w = {}
        for nm in ("qhi", "khi", "qlo", "klo", "vhi", "vlo"):
            t_w[nm] = wpool.tile([128, GT, 2, CC], FP8, tag=f"w{nm}", name=f"w{nm}")
        # weight DMAs are emitted on the SP queue interleaved with the first
        # panel in NEED order (see the bi == 0 block below): the shared DMA
        # device serves ready transfers in order, so queue order = priority.
        t_cs = consts.tile([128, 2, S], BF16, tag="cs")   # DMA'd on SP below

        ident_bf = consts.tile([128, 128], BF16, tag="ident_bf")
        make_identity(nc, ident_bf)
        # PE warm-up: dep-free junk matmuls so the p-state ramp hits peak
        # clock before the first real chain (and the DMA-bound startup
        # window never resets it).
        warm = psum_l.tile([128, 128], F32, tag="l", name="warm")
        for _ in range(42):
            nc.tensor.matmul(warm, ident_bf, ident_bf, start=True, stop=True,
                             skip_group_check=True)
        # [I, 0, I] in fp8: id8[:, 0:2] = [I;0] (even k-tile), id8[:, 1:3] = [0;I]
        # host-loaded: on-device fp8 init breaks the walrus backend.
        # (DMAs deferred past the startup window: needed only in phase B.)
        id8 = consts.tile([128, 3, 128], FP8, tag="id8")
        t_woh = consts.tile([128, HPC, D], FP8, tag="woh")
        t_wol = consts.tile([128, HPC, D], FP8, tag="wol")
        t_logw = consts.tile([128, B, ST], F32, tag="logw")

        # ---------------- helpers ------------------------------------------
        panel_cache = {}

        def emit_panel(b, sb_i, split=False):
            blk = slice(sb_i * SB, sb_i * SB + SB)
            phi = panels.tile([128, GT, 2, SB], FP8, tag="phi", name="phi")
            plo = panels.tile([128, GT, 2, SB], FP8, tag="plo", name="plo")
            if split:  # halve the first transfers so chains start early
                nc.sync.dma_start(phi[:, 0:GT // 2], d_xhi[b, :, 0:GT // 2, :, blk])
                nc.sync.dma_start(phi[:, GT // 2:], d_xhi[b, :, GT // 2:, :, blk])
                nc.sync.dma_start(plo[:, 0:GT // 2], d_xlo[b, :, 0:GT // 2, :, blk])
                nc.sync.dma_start(plo[:, GT // 2:], d_xlo[b, :, GT // 2:, :, blk])
            else:
                nc.sync.dma_start(phi, d_xhi[b, :, :, :, blk])
                nc.sync.dma_start(plo, d_xlo[b, :, :, :, blk])
            if b == 0:
                # cs chunk for this block, after the panel: RoPE needs it
                # later than the matmul chains need the panel.
                nc.sync.dma_start(t_cs[:, :, blk], d_cs[:, :, blk])
            return phi, plo

        def dr_part(ps, terms, start, stop, gr=None):
            gr = gr if gr is not None else range(GT)
            n = len(terms) * len(gr)
            idx = 0
            for wt, xt, hc in terms:
                for g in gr:
                    nc.tensor.matmul(
                        ps, wt[:, g, :, hc], xt[:, g, :, :],
                        start=(start and idx == 0),
                        stop=(stop and idx == n - 1),
                        perf_mode=DR, skip_group_check=True)
                    idx += 1

        def dr_chain(ps, whi, wlo, phi, plo, hc):
            # sum of 3 fp8 DoubleRow cross terms; hi*hi first so the chain
            # can start before the lo tensors arrive.
            dr_part(ps, [(whi, phi, hc), (whi, plo, hc), (wlo, phi, hc)],
                    True, True)

        ncp = [0]
        pending_y = []

        def flush_y(k=1):
            # y-DMAs are emitted one C-unit group late so the (in-order)
            # issuing SEQ never parks on a not-yet-copied staging tile.
            # All y traffic rides the SP queue: parking a DMA on the ACT or
            # DVE queues would stall exp / U-adds queued behind it.
            for _ in range(k):
                if pending_y:
                    dst, ys = pending_y.pop(0)
                    nc.sync.dma_start(dst, ys)

        draining = [False]

        # y staging: [128, 4, 512] bf16 per (b, st); one DMA per 4 C-units.
        ystage = {}

        def emit_c_unit(b, st, nb, oh, ol, split_dma=False):
            ss = slice(st * 128, (st + 1) * 128)
            ns = slice(nb * 512, (nb + 1) * 512)
            # during the final drain the score pool is idle: borrow its banks
            # to deepen the C-unit pipeline.
            if draining[0] and ncp[0] % 4 == 0:
                ops = psum_mm.tile([128, 512], F32, tag="mm", name="ops")
            elif draining[0] and ncp[0] % 4 == 1:
                ops = psum_pv.tile([128, 512], F32, tag="pv", name="ops")
            elif draining[0] and ncp[0] % 4 == 2:
                ops = psum_l.tile([128, 512], F32, tag="l", name="ops")
            else:
                ops = psum_c.tile([128, 512], F32, tag="c", name="ops")
            # 3 fp8 DoubleRow cross terms over both heads (256 channels)
            for i, (o_t, w_t) in enumerate(((oh, t_woh), (oh, t_wol),
                                            (ol, t_woh))):
                nc.tensor.matmul(
                    ops, o_t[:, :, ss], w_t[:, :, ns],
                    start=(i == 0), stop=(i == 2), perf_mode=DR,
                    skip_group_check=True)
            if nb == 0:
                ystage[(b, st)] = outp.tile([128, 4, 512], BF16, tag="ys", name="ys")
            ys = ystage[(b, st)]
            ncp[0] += 1
            if ncp[0] % 2 == 0:
                nc.scalar.activation(ys[:, nb, :], ops, CPY, 0.0,
                                     1.0 / (WSC * OSC))
            else:
                nc.vector.tensor_scalar_mul(ys[:, nb, :], ops,
                                            1.0 / (WSC * OSC))
            if split_dma:
                # end-critical tiles: one DMA per quarter, rotated across
                # queues so the final drain's DMA issues overlap
                eng = (nc.sync, nc.scalar, nc.sync, nc.gpsimd)[ncp[0] % 4]
                eng.dma_start(d_y[b, st, :, ns], ys[:, nb, :])
            elif nb == 3:
                pending_y.append((d_y[b, st], ys))
                flush_y(1)

        slab_cache = {}

        def emit_slab(h, j):
            # merged slab for all k-tile pairs of (h, j): [128, 2(j+1), 2, QB]
            gmax = 2 * (j + 1)
            qs = slice(j * QB, (j + 1) * QB)
            slab = slabs.tile([128, 8, 2, QB], FP8, tag="slab")
            nc.sync.dma_start(
                slab[:, 0:gmax],
                d_bias[h, 0:gmax * 256, qs].rearrange(
                    "(g r p) q -> p g r q", p=128, r=2))
            return slab

        # deferred work queue: (kind, closure) entries — v-transposes,
        # normalization tails and C-units — that fill PE gaps in later
        # m-loops. A tail's chain waits on the previous block's exp/U work,
        # so tails are only popped when `late` (the consumer has caught up);
        # C-units never jump ahead of their own block's tails.
        fill_q = []
        pending_fin = []   # deferred last-PV + tail of the previous m-loop
        popc = [0]         # pop_fill call counter (for entry locks)

        def flush_fin():
            while pending_fin:
                pending_fin.pop(0)()

        def pop_fill(k=1, late=True):
            # entries carry a lock (pop-call count before which they must not
            # run): a C-unit popped too soon stalls the PE on its tail's
            # oT writes.
            popc[0] += 1
            n = 0
            for _ in range(k):
                if not fill_q or (not draining[0]
                                  and fill_q[0][1] > popc[0]):
                    return n
                fill_q.pop(0)[2]()
                n += 1
            return n

        # ============ phase A: x^T panels -> q^T,k^T (RoPE), v ============
        qT = {}
        kT = {}
        vv = {}
        for b in range(B):
            for h in range(HPC):
                qT[b, h] = qkv.tile([128, S], BF16, tag=f"qT{b}{h}", name="qT")
                kT[b, h] = qkv.tile([128, S], BF16, tag=f"kT{b}{h}", name="kT")
                vv[b, h] = qkv.tile([128, ST, HD], BF16, tag=f"v{b}{h}", name="vv")

        def rope_emit(ps, dest, blk):
            # stage PSUM f32 -> SBUF bf16 once (ACT), then all-SBUF bf16
            # muls/adds run in the DVE's fast mode.
            # cs slot 0 = [cosT;cosT]/64, slot 1 = [-sinT;+sinT]/64
            prs = rope.tile([128, SB], BF16, tag="prs")
            nc.scalar.copy(prs, ps)
            t1 = rope.tile([128, SB], BF16, tag="t1")
            t2 = rope.tile([128, SB], BF16, tag="t2")
            nc.vector.tensor_mul(t1, prs, t_cs[:, 0, blk])
            nc.vector.tensor_mul(
                t2[0:64, :], prs[64:128, :], t_cs[0:64, 1, blk])
            nc.vector.tensor_mul(
                t2[64:128, :], prs[0:64, :], t_cs[64:128, 1, blk])
            nc.vector.tensor_add(dest[:, blk], t1, t2)

        def make_vtrans(b, h, sb_i, vts):
            # flip vT [hd, s] -> v [s, hd] with 4 PE transposes + one copy
            def vtrans():
                vtp = psum_l.tile([128, 4, 128], BF16, tag="l", name="vtp")
                for c4 in range(4):
                    cs4 = slice(c4 * 128, (c4 + 1) * 128)
                    nc.tensor.transpose(vtp[:, c4, :], vts[:, cs4], ident_bf)
                dstv = vv[b, h][:, sb_i * 4:(sb_i + 1) * 4, :]
                if ncp[0] % 2 == 0:
                    nc.scalar.copy(dstv, vtp)
                else:
                    nc.vector.tensor_copy(dstv, vtp)
                ncp[0] += 1
            return vtrans

        vq = []  # pending v-transposes (popped at the next block)

        for bi, (b, sb_i) in enumerate([(b, s) for b in range(B)
                                        for s in range(NSB)]):
            s0 = sb_i * SB
            blk = slice(s0, s0 + SB)
            if (b, sb_i) in panel_cache:
                phi, plo = panel_cache.pop((b, sb_i))
            elif bi == 0:
                # cold start: interleave the first panel's halves with the
                # weight tiles in consumption order
                blk0 = slice(0, SB)
                phi = panels.tile([128, GT, 2, SB], FP8, tag="phi", name="phi")
                plo = panels.tile([128, GT, 2, SB], FP8, tag="plo", name="plo")
                nc.sync.dma_start(phi[:, 0:GT // 2], d_xhi[0, :, 0:GT // 2, :, blk0])
                nc.sync.dma_start(phi[:, GT // 2:], d_xhi[0, :, GT // 2:, :, blk0])
                for nm in ("qhi", "khi"):
                    nc.sync.dma_start(t_w[nm][:, 0:GT // 2], d_w[nm][:, 0:GT // 2])
                nc.sync.dma_start(plo[:, 0:GT // 2], d_xlo[0, :, 0:GT // 2, :, blk0])
                nc.sync.dma_start(plo[:, GT // 2:], d_xlo[0, :, GT // 2:, :, blk0])
                for nm in ("qhi", "khi"):
                    nc.sync.dma_start(t_w[nm][:, GT // 2:], d_w[nm][:, GT // 2:])
                nc.sync.dma_start(t_cs[:, :, blk0], d_cs[:, :, blk0])
                for nm in ("qlo", "klo"):
                    nc.sync.dma_start(t_w[nm], d_w[nm])
            else:
                phi, plo = emit_panel(b, sb_i)
            # prefetch next block's panel while this one computes
            nxt = [(bb, ss) for bb in range(B) for ss in range(NSB)][bi + 1:bi + 2]
            if nxt and nxt[0] not in panel_cache:
                panel_cache[nxt[0]] = emit_panel(*nxt[0])
            if bi == 0:
                # v weights after the next panel: consumed later than it
                for nm in ("vhi", "vlo"):
                    nc.sync.dma_start(t_w[nm], d_w[nm])

            # q,k chains (both heads) first, then v: the first v chain
            # then starts after wv has streamed in.
            first = bi == 0
            if first:
                # cold start: hi*hi parts of all four q/k chains first
                # (split by panel half), so the PE works while the lo
                # tensors are still streaming in.
                chains = [(pre, h) for h in range(HPC) for pre in ("q", "k")]
                pss = {}
                for i, (pre, h) in enumerate(chains):
                    hc = slice(h * HD, (h + 1) * HD)
                    pool, tag = ((psum_mm, "mm") if i < 3 else
                                 (psum_c, "c"))
                    ps = pool.tile([128, SB], F32, tag=tag, name="pss")
                    pss[(pre, h)] = ps
                    dr_part(ps, [(t_w[pre + "hi"], phi, hc)], True, False,
                            gr=range(GT // 2))
                for pre, h in chains:
                    hc = slice(h * HD, (h + 1) * HD)
                    dr_part(pss[(pre, h)], [(t_w[pre + "hi"], phi, hc)],
                            False, False, gr=range(GT // 2, GT))
                for pre, h in chains:
                    hc = slice(h * HD, (h + 1) * HD)
                    dr_part(pss[(pre, h)], [(t_w[pre + "hi"], plo, hc)],
                            False, False)
                for pre, h in chains:
                    hc = slice(h * HD, (h + 1) * HD)
                    ps = pss[(pre, h)]
                    dr_part(ps, [(t_w[pre + "lo"], phi, hc)], False, True)
                    rope_emit(ps, (qT if pre == "q" else kT)[b, h], blk)
            else:
                for ci, (h, pre) in enumerate(
                        [(h, p) for h in range(HPC) for p in ("q", "k")]):
                    hc = slice(h * HD, (h + 1) * HD)
                    ps = psum_mm.tile([128, SB], F32, tag="mm")
                    dr_chain(ps, t_w[pre + "hi"], t_w[pre + "lo"],
                             phi, plo, hc)
                    rope_emit(ps, (qT if pre == "q" else kT)[b, h], blk)
                    if ci == 0 and vq:
                        # previous block's v-transposes: the staging copy
                        # they wait on has finished during this chain
                        for f in vq:
                            f()
                        vq.clear()
            if first and vq:
                pass
            # v chains, transposed like q/k, then staged to SBUF bf16
            for h in range(HPC):
                hc = slice(h * HD, (h + 1) * HD)
                ps = psum_mm.tile([128, SB], F32, tag="mm")
                dr_chain(ps, t_w["vhi"], t_w["vlo"], phi, plo, hc)
                vts = vtsp.tile([128, SB], BF16, tag="vts")
                nc.scalar.activation(vts, ps, CPY, 0.0, 1.0 / WSC)
                vq.append(make_vtrans(b, h, sb_i, vts))
            if bi == 0:
                # on ACT, not Pool: any DMA in Pool's stream lands before
                # make_identity in the scheduled order and the warm-up
                # fillers' semaphore would wait on its completion.
                nc.sync.dma_start(t_logw, d_logw)
                nc.sync.dma_start(id8, d_id8)
            if bi == 6:
                # prefetch phase B's first slabs
                slab_cache[(0, 0)] = emit_slab(0, 0)
                slab_cache[(1, 0)] = emit_slab(1, 0)
            if bi == 7:
                nc.sync.dma_start(t_woh, d_woh)
                nc.sync.dma_start(t_wol, d_wol)
        # last block's v-transposes run inside the first m-loop (fill_q)
        for f in vq:
            fill_q.append(("v", 0, f))
        vq = []

        # attn output o^T, fp8 hi/lo, heads packed on the DoubleRow pair
        # axis for the output projection: [128(hd), HPC, S]
        oT = {}
        for b in range(B):
            oT[b, "h"] = qkv.tile([128, HPC, S], FP8, tag=f"oh{b}", name="oh")
            oT[b, "l"] = qkv.tile([128, HPC, S], FP8, tag=f"ol{b}", name="ol")

        # ====== phases B+C software-pipelined over q-blocks, batch-inner ======
        for j in range(NQB):
            qs = slice(j * QB, (j + 1) * QB)
            nk = 4 * (j + 1)          # causal: k-tiles 0..nk-1
            for h in range(HPC):
                slab = slab_cache.pop((h, j), None)
                if slab is None:
                    slab = emit_slab(h, j)
                # prefetch the next (h, j) slab
                if h + 1 < HPC:
                    if (h + 1, j) not in slab_cache:
                        slab_cache[(h + 1, j)] = emit_slab(h + 1, j)
                elif j + 1 < NQB and (0, j + 1) not in slab_cache:
                    slab_cache[(0, j + 1)] = emit_slab(0, j + 1)
                for b in range(B):
                    pv = psum_pv.tile([128, QB], F32, tag="pv", name="pv")
                    U = upool.tile([128, QB], BF16, tag="U", name="U")
                    offs = [max(0, (m - 4 * j) * 128) for m in range(nk)]

                    def emit_pvu(m, ex, pv=pv, U=U, b=b, h=h, nk=nk,
                                 offs=offs):
                        off = offs[m]
                        nc.tensor.matmul(
                            pv[:, off:], vv[b, h][:, m, :], ex[:, off:],
                            start=(m == 0), stop=(m == nk - 1),
                            skip_group_check=True)
                        if m == 0:
                            nc.vector.tensor_copy(U, ex)
                        else:
                            nc.vector.tensor_add(
                                U[:, off:], U[:, off:], ex[:, off:])

                    # software-skewed m loop: scores/bias/exp of m, then the
                    # PV+U of m-1 (covering the exp latency with fill pops).
                    # The last PV + normalization tail are deferred into the
                    # NEXT m-loop (emitted behind its first QK+bias), so the
                    # final exp's latency never stalls the PE.
                    prev = None
                    sc_next = None
                    if not fill_q and not pending_fin:
                        sc_next = psum_mm.tile([128, QB], F32, tag="mm",
                                               name="sc_next")
                        for _ in range(3 if j == 0 and h == 0 else 2):
                            nc.tensor.matmul(
                                sc_next, ident_bf, qT[b, h][:, qs],
                                start=True, stop=True, skip_group_check=True)
                    for m in range(nk):
                        ml = m % 2
                        off = offs[m]
                        qso = slice(j * QB + off, (j + 1) * QB)
                        if sc_next is not None:
                            sc = sc_next
                            sc_next = None
                        else:
                            sc = psum_mm.tile([128, QB], F32, tag="mm")
                        nc.tensor.matmul(
                            sc[:, off:], kT[b, h][:, m * 128:(m + 1) * 128],
                            qT[b, h][:, qso],
                            start=True, stop=False)
                        nc.tensor.matmul(
                            sc[:, off:], id8[:, ml:ml + 2, :],
                            slab[:, m // 2, :, off:],
                            start=False, stop=True, perf_mode=DR,
                            skip_group_check=True)
                        ex = expp.tile([128, QB], BF16, tag="ex")
                        nc.scalar.activation(ex[:, off:], sc[:, off:], EXP,
                                             bias=t_logw[:, b, m:m + 1])
                        if m == 0:
                            flush_fin()
                        if prev is not None:
                            if b == B - 1 and j == NQB - 1 and h == HPC - 1:
                                # keep a few C-units back: they fill the PE
                                # during the final tail's serial chain
                                k = 1 if m % 2 else 0
                            else:
                                k = 2 if len(fill_q) > 8 else 1
                            pop_fill(k, late=(m >= 3))
                            emit_pvu(*prev)
                        prev = (m, ex)

                    last = (b == B - 1 and j == NQB - 1 and h == HPC - 1)

                    def make_fin(prev=prev, pv=pv, U=U, b=b, h=h, qs=qs,
                                 j=j, last=last, emit_pvu=emit_pvu):
                        def fin():
                            emit_pvu(*prev)
                            # tail: no PE work (partition_all_reduce replaces
                            # the ones-matmul); runs on Pool/DVE/ACT behind
                            # the current m-loop. Row sums (all partitions),
                            # reciprocal, normalize, fp8 hi/lo of o^T * 64.
                            if last:
                                # end-critical: chunk the chain so the first
                                # drain C-units (one 128-col oT slice each)
                                # start before the later chunks normalize
                                chunks = [slice(c * 128, (c + 1) * 128)
                                          for c in range(QB // 128)]
                            else:
                                chunks = [slice(0, QB)]
                            L = normp.tile([128, QB], F32, tag="L")
                            rb = normp.tile([128, QB], F32, tag="rb")
                            t = normp.tile([128, QB], BF16, tag="t")
                            for cs4 in chunks:
                                qc = slice(qs.start + cs4.start,
                                           qs.start + cs4.stop)
                                nc.gpsimd.partition_all_reduce(
                                    L[:, cs4], U[:, cs4], 128, RADD)
                                nc.vector.reciprocal(rb[:, cs4], L[:, cs4])
                                nc.vector.tensor_mul(
                                    t[:, cs4], pv[:, cs4], rb[:, cs4])
                                nc.scalar.activation(
                                    oT[b, "h"][:, h, qc], t[:, cs4], CPY,
                                    0.0, OSC)
                                nc.gpsimd.scalar_tensor_tensor(
                                    oT[b, "l"][:, h, qc], t[:, cs4], OSC,
                                    oT[b, "h"][:, h, qc], MUL, SUB)
                            if h == HPC - 1:
                                # both heads of (b, j) normalized: its
                                # C-units can pop once the tail chain has
                                # had ~10 pop-calls of headroom
                                fill_q.extend(
                                    ("c", popc[0] + 10,
                                     lambda st=st, nb=nb, b=b:
                                     emit_c_unit(
                                         b, st, nb, oT[b, "h"], oT[b, "l"],
                                         split_dma=(j == NQB - 1)))
                                    for st in range(4 * j, 4 * j + 4)
                                    for nb in range(4))
                        return fin

                    pending_fin.append(make_fin())

        flush_fin()
        draining[0] = True
        while fill_q:
            fill_q.pop(0)[2]()
        flush_y(len(pending_y))

    nc.compile()
    return nc


def _host_prep(x, Wq, Wk, Wv, Wo, policy_mask, memory_weights):
    """Build the per-core input maps."""
    bf = ml_dtypes.bfloat16
    f8 = ml_dtypes.float8_e4m3

    def hilo(a):
        hi = a.astype(f8)
        lo = (a - hi.astype(np.float32)).astype(f8)
        return hi, lo

    def hilo_tiles(a):
        # [D, C] (or [D, S]) -> hi/lo fp8 in [128, GT, 2, C] DoubleRow layout
        hi, lo = hilo(a)
        def tl(t):
            return np.ascontiguousarray(
                t.reshape(GT, 2, 128, -1).transpose(2, 0, 1, 3))
        return tl(hi), tl(lo)

    xhi = np.empty((B, 128, GT, 2, S), f8)
    xlo = np.empty((B, 128, GT, 2, S), f8)
    for b in range(B):
        xt = np.ascontiguousarray(np.asarray(x[b], np.float32).T)  # [D, S]
        xhi[b], xlo[b] = hilo_tiles(xt)

    # RoPE tables (carry the 1/WSC weight descale):
    inv_freq = (1.0 / (ROPE_BASE ** (np.arange(0, HD, 2, dtype=np.float32) / HD)))
    t = np.arange(S, dtype=np.float32)
    freqs = np.outer(t, inv_freq).astype(np.float32)      # [S, 64]
    cosT = np.cos(freqs).T.astype(np.float32) / WSC       # [64, S]
    sinT = np.sin(freqs).T.astype(np.float32) / WSC
    cs = np.empty((128, 2, S), np.float32)
    cs[0:64, 0] = cosT
    cs[64:128, 0] = cosT
    cs[0:64, 1] = -sinT
    cs[64:128, 1] = sinT
    cs = cs.astype(bf)

    # memory multiplier w = 1 + GS*mw + 1e-8  (exp(log1p(z)) = 1+z);
    # exact f32, applied as the exp's per-partition bias: [128, B, ST]
    mw = memory_weights.reshape(B, S).astype(np.float64)
    logw = np.log(1.0 + GS * mw + 1e-8).astype(np.float32)  # [B, S]
    logw_t = np.ascontiguousarray(
        logw.reshape(B, ST, 128).transpose(2, 0, 1))        # [128, B, ST]

    # transposed, causal-masked, pre-scaled policy bias per head
    # (batch-independent: log w rides the exp bias operand instead)
    maskT = np.tril(np.full((S, S), MASK_NEG, dtype=np.float32), -1)
    pol = np.asarray(policy_mask, dtype=np.float32)[0]    # [H, S, S]

    id8h = np.zeros((128, 3, 128), np.float32)
    id8h[:, 0, :] = np.eye(128, dtype=np.float32)
    id8h[:, 2, :] = np.eye(128, dtype=np.float32)
    id8h = id8h.astype(f8)

    in_maps = []
    for c in range(NCORES):
        cols = slice(c * HPC * HD, (c + 1) * HPC * HD)
        bias_c = np.empty((HPC, S, S), dtype=f8)
        for hl in range(HPC):
            hg = c * HPC + hl
            bias_c[hl] = (GS * pol[hg].T + maskT).astype(f8)
        wo_c = np.ascontiguousarray(
            np.asarray(Wo, np.float32)[cols, :]
            .reshape(HPC, 128, D).transpose(1, 0, 2)) * np.float32(WSC)
        woh, wol = hilo(wo_c)
        m = {"xhi": xhi, "xlo": xlo, "woh": woh, "wol": wol,
             "biasT": bias_c, "cs": cs, "id8": id8h, "logw": logw_t}
        for nm, w, s in (("q", Wq, WSC), ("k", Wk, WSC * SCALE), ("v", Wv, WSC)):
            hi, lo = hilo_tiles(np.asarray(w, np.float32)[:, cols] * np.float32(s))
            m[f"w{nm}hi"] = hi
            m[f"w{nm}lo"] = lo
        in_maps.append(m)
    return in_maps


def kernel(x, Wq, Wk, Wv, Wo, bo, policy_mask, memory_weights):
    x = np.asarray(x, dtype=np.float32)
    Wq = np.asarray(Wq, dtype=np.float32)
    Wk = np.asarray(Wk, dtype=np.float32)
    Wv = np.asarray(Wv, dtype=np.float32)
    Wo = np.asarray(Wo, dtype=np.float32)
    bo = np.asarray(bo, dtype=np.float32)

    if "nc" not in _CACHE:
        _CACHE["nc"] = build_nc()
    nc = _CACHE["nc"]

    in_maps = _host_prep(x, Wq, Wk, Wv, Wo, policy_mask, memory_weights)
    res = run_bass_kernel_spmd(nc, in_maps, core_ids=list(range(NCORES)))

    acc = np.zeros((B, S, D), dtype=np.float64)
    for c in range(NCORES):
        acc += res.results[c]["y"].astype(np.float64).reshape(B, S, D)
    return (acc + bo.astype(np.float64)).astype(np.float32)


# revision 15
# speedup vs baseline: 1.0313x; 1.0313x over previous
"""EnhancedGovernanceAttention Trainium2 kernel (8 NeuronCores, SPMD).

Sharding: core c owns heads {2c, 2c+1} for BOTH batches. Each core computes
its heads' attention and a row-parallel partial of the Wo projection; the
host sums the 8 partials and adds bo.

Math notes (vs the jax reference):
 - softmax max-subtraction is dropped: scores ~ N(0,1) + bias in [0,0.3],
   so exp() cannot overflow; softmax is shift-invariant.
 - log1p memory bias log(1 + GS*mw + 1e-8) = log(w) is per-(batch,key):
   it rides the exp's per-partition bias operand (exact f32), so
   exp(score+logw) carries w for both the PV numerator and the row-sum.
 - the remaining bias (GS*policy + causal -40 mask) is batch-INDEPENDENT:
   phase B runs batch-inner so each fp8 slab is DMA'd once and used by
   both batches' m-loops.
 - causal mask: only lower-triangle k-tiles are computed; the intra-tile
   diagonal mask is baked into the (fp8) bias as -40.
 - scores are computed TRANSPOSED ([k, q]) so the PV matmul directly
   yields attn outputs o^T = (PV)^T/l, which is the lhsT the output
   projection needs.
 - QKV projections run as fp8 DoubleRow matmuls on a hi/lo split of x and
   64*W (3 cross terms; the 64x pre-scale keeps the lo residuals out of
   fp8's subnormal range; 1/64 is folded into the RoPE tables and the V
   staging copy). V is computed transposed like q/k (wide chains, ~4x
   fewer PE instructions) and flipped to [s, hd] with PE transposes.
 - the policy bias is added into the score PSUM with an fp8 DoubleRow
   identity matmul (2 k-tiles per slab pair, [I;0]/[0;I] selects the slot).
 - softmax denominator: exp tiles accumulate into U on the DVE; one Pool
   partition_all_reduce per (b,h,q-block) yields the row sums broadcast
   to all partitions (no ones-matmul, no separate broadcast).
 - output projection in fp8 DoubleRow: the tail splits o^T (x64) into
   fp8 hi+lo; C-units contract BOTH heads (256 ch) per instruction with
   3 cross terms against 64*Wo hi/lo; the y staging copy descales 1/4096.
 - y is written as contiguous [B, ST, 128, D] tiles (one descriptor run
   per DMA, 4 C-units batched per DMA); host reassembles.
"""

import numpy as np
import ml_dtypes
from contextlib import ExitStack

import concourse.bass as bass
import concourse.tile as tile
from concourse import bacc, bass_isa, mybir
from concourse.bass_utils import run_bass_kernel_spmd
from concourse.masks import make_identity

B, S, D, H, HD = 2, 2048, 2048, 16, 128
GS = 0.1
ROPE_BASE = 10000.0
NCORES = 8
HPC = H // NCORES          # heads per core = 2
SCALE = float(HD) ** -0.5
DT = D // 128              # 16 d-tiles
GT = DT // 2               # 8 d-tile pairs (DoubleRow)
ST = S // 128              # 16 s-tiles (also k-tiles)
QB = 512                   # q-block width (phase B)
NQB = S // QB              # 4 q-blocks
SB = 512                   # s-block width (phase A panels)
NSB = S // SB              # 4 s-blocks
MASK_NEG = -40.0
WSC = 64.0                 # fp8 weight pre-scale
OSC = 64.0                 # fp8 attn-output pre-scale

F32 = mybir.dt.float32
BF16 = mybir.dt.bfloat16
FP8 = mybir.dt.float8e4
EXP = mybir.ActivationFunctionType.Exp
CPY = mybir.ActivationFunctionType.Copy
DR = mybir.MatmulPerfMode.DoubleRow
MUL = mybir.AluOpType.mult
SUB = mybir.AluOpType.subtract
RADD = bass_isa.ReduceOp.add

_CACHE = {}


def build_nc():
    nc = bacc.Bacc("TRN2", target_bir_lowering=False, debug=False,
                   num_devices=NCORES)

    d_xhi = nc.dram_tensor("xhi", [B, 128, GT, 2, S], FP8, kind="ExternalInput").ap()
    d_xlo = nc.dram_tensor("xlo", [B, 128, GT, 2, S], FP8, kind="ExternalInput").ap()
    CC = HPC * HD
    d_w = {}
    for nm in ("qhi", "qlo", "khi", "klo", "vhi", "vlo"):
        d_w[nm] = nc.dram_tensor(f"w{nm}", [128, GT, 2, CC], FP8,
                                 kind="ExternalInput").ap()
    d_woh = nc.dram_tensor("woh", [128, HPC, D], FP8, kind="ExternalInput").ap()
    d_wol = nc.dram_tensor("wol", [128, HPC, D], FP8, kind="ExternalInput").ap()
    d_id8 = nc.dram_tensor("id8", [128, 3, 128], FP8, kind="ExternalInput").ap()
    d_bias = nc.dram_tensor("biasT", [HPC, S, S], FP8, kind="ExternalInput").ap()
    d_cs = nc.dram_tensor("cs", [128, 2, S], BF16, kind="ExternalInput").ap()
    d_logw = nc.dram_tensor("logw", [128, B, ST], F32, kind="ExternalInput").ap()
    d_y = nc.dram_tensor("y", [B, ST, 128, D], BF16, kind="ExternalOutput").ap()

    with tile.TileContext(nc) as tc, ExitStack() as ctx:
        consts = ctx.enter_context(tc.tile_pool(name="consts", bufs=1))
        wpool = ctx.enter_context(tc.tile_pool(name="wpool", bufs=1))
        qkv = ctx.enter_context(tc.tile_pool(name="qkv", bufs=1))
        panels = ctx.enter_context(tc.tile_pool(name="panels", bufs=2))
        rope = ctx.enter_context(tc.tile_pool(name="rope", bufs=3))
        vtsp = ctx.enter_context(tc.tile_pool(name="vtsp", bufs=2))
        slabs = ctx.enter_context(tc.tile_pool(name="slabs", bufs=2))
        expp = ctx.enter_context(tc.tile_pool(name="expp", bufs=4))
        upool = ctx.enter_context(tc.tile_pool(name="upool", bufs=3))
        normp = ctx.enter_context(tc.tile_pool(name="normp", bufs=2))
        outp = ctx.enter_context(tc.tile_pool(name="outp", bufs=4))
        psum_mm = ctx.enter_context(tc.tile_pool(name="psum_mm", bufs=3, space="PSUM"))
        psum_pv = ctx.enter_context(tc.tile_pool(name="psum_pv", bufs=2, space="PSUM"))
        psum_c = ctx.enter_context(tc.tile_pool(name="psum_c", bufs=2, space="PSUM"))
        psum_l = ctx.enter_context(tc.tile_pool(name="psum_l", bufs=1, space="PSUM"))

        # ---------------- constants (emission order = queue priority) ----------
        # ACT queue: q/k weights first (first chains need them); halves so the
        # cold-start chains can begin on the first half.
        t_w = {}
        for nm in ("qhi", "khi", "qlo", "klo", "vhi", "vlo"):
            t_w[nm] = wpool.tile([128, GT, 2, CC], FP8, tag=f"w{nm}", name=f"w{nm}")
        # weight DMAs are emitted on the SP queue interleaved with the first
        # panel in NEED order (see the bi == 0 block below): the shared DMA
        # device serves ready transfers in order, so queue order = priority.
        t_cs = consts.tile([128, 2, S], BF16, tag="cs")   # DMA'd on SP below

        ident_bf = consts.tile([128, 128], BF16, tag="ident_bf")
        make_identity(nc, ident_bf)
        # PE warm-up: dep-free junk matmuls so the p-state ramp hits peak
        # clock before the first real chain (and the DMA-bound startup
        # window never resets it).
        warm = psum_l.tile([128, 128], F32, tag="l", name="warm")
        for _ in range(42):
            nc.tensor.matmul(warm, ident_bf, ident_bf, start=True, stop=True,
                             skip_group_check=True)
        # [I, 0, I] in fp8: id8[:, 0:2] = [I;0] (even k-tile), id8[:, 1:3] = [0;I]
        # host-loaded: on-device fp8 init breaks the walrus backend.
        # (DMAs deferred past the startup window: needed only in phase B.)
        id8 = consts.tile([128, 3, 128], FP8, tag="id8")
        t_woh = consts.tile([128, HPC, D], FP8, tag="woh")
        t_wol = consts.tile([128, HPC, D], FP8, tag="wol")
        t_logw = consts.tile([128, B, ST], F32, tag="logw")

        # ---------------- helpers ------------------------------------------
        panel_cache = {}

        def emit_panel(b, sb_i, split=False):
            blk = slice(sb_i * SB, sb_i * SB + SB)
            phi = panels.tile([128, GT, 2, SB], FP8, tag="phi", name="phi")
            plo = panels.tile([128, GT, 2, SB], FP8, tag="plo", name="plo")
            if split:  # halve the first transfers so chains start early
                nc.sync.dma_start(phi[:, 0:GT // 2], d_xhi[b, :, 0:GT // 2, :, blk])
                nc.sync.dma_start(phi[:, GT // 2:], d_xhi[b, :, GT // 2:, :, blk])
                nc.sync.dma_start(plo[:, 0:GT // 2], d_xlo[b, :, 0:GT // 2, :, blk])
                nc.sync.dma_start(plo[:, GT // 2:], d_xlo[b, :, GT // 2:, :, blk])
            else:
                nc.sync.dma_start(phi, d_xhi[b, :, :, :, blk])
                nc.sync.dma_start(plo, d_xlo[b, :, :, :, blk])
            if b == 0:
                # cs chunk for this block, after the panel: RoPE needs it
                # later than the matmul chains need the panel.
                nc.sync.dma_start(t_cs[:, :, blk], d_cs[:, :, blk])
            return phi, plo

        def dr_part(ps, terms, start, stop, gr=None):
            gr = gr if gr is not None else range(GT)
            n = len(terms) * len(gr)
            idx = 0
            for wt, xt, hc in terms:
                for g in gr:
                    nc.tensor.matmul(
                        ps, wt[:, g, :, hc], xt[:, g, :, :],
                        start=(start and idx == 0),
                        stop=(stop and idx == n - 1),
                        perf_mode=DR, skip_group_check=True)
                    idx += 1

        def dr_chain(ps, whi, wlo, phi, plo, hc):
            # sum of 3 fp8 DoubleRow cross terms; hi*hi first so the chain
            # can start before the lo tensors arrive.
            dr_part(ps, [(whi, phi, hc), (whi, plo, hc), (wlo, phi, hc)],
                    True, True)

        ncp = [0]
        pending_y = []

        def flush_y(k=1):
            # y-DMAs are emitted one C-unit group late so the (in-order)
            # issuing SEQ never parks on a not-yet-copied staging tile.
            # All y traffic rides the SP queue: parking a DMA on the ACT or
            # DVE queues would stall exp / U-adds queued behind it.
            for _ in range(k):
                if pending_y:
                    dst, ys = pending_y.pop(0)
                    nc.sync.dma_start(dst, ys)

        draining = [False]

        # y staging: [128, 4, 512] bf16 per (b, st); one DMA per 4 C-units.
        ystage = {}

        def emit_c_unit(b, st, nb, oh, ol, split_dma=False):
            ss = slice(st * 128, (st + 1) * 128)
            ns = slice(nb * 512, (nb + 1) * 512)
            # during the final drain the score pool is idle: borrow its banks
            # to deepen the C-unit pipeline. In phase B, psum_l is idle and
            # serves as the C-units' third bank (the staging copy that
            # releases a bank can lag ~1us behind a busy vector queue).
            if draining[0] and ncp[0] % 4 == 0:
                ops = psum_mm.tile([128, 512], F32, tag="mm", name="ops")
            elif draining[0] and ncp[0] % 4 == 1:
                ops = psum_pv.tile([128, 512], F32, tag="pv", name="ops")
            elif ncp[0] % 3 == 2:
                ops = psum_l.tile([128, 512], F32, tag="l", name="ops")
            else:
                ops = psum_c.tile([128, 512], F32, tag="c", name="ops")
            # 3 fp8 DoubleRow cross terms over both heads (256 channels)
            for i, (o_t, w_t) in enumerate(((oh, t_woh), (oh, t_wol),
                                            (ol, t_woh))):
                nc.tensor.matmul(
                    ops, o_t[:, :, ss], w_t[:, :, ns],
                    start=(i == 0), stop=(i == 2), perf_mode=DR,
                    skip_group_check=True)
            if nb == 0:
                ystage[(b, st)] = outp.tile([128, 4, 512], BF16, tag="ys", name="ys")
            ys = ystage[(b, st)]
            ncp[0] += 1
            if draining[0]:
                # ACT is free during the drain; in phase B it is the exp
                # engine and a ys copy parked there stalls the m-loop
                if ncp[0] % 2 == 0:
                    nc.scalar.activation(ys[:, nb, :], ops, CPY, 0.0,
                                         1.0 / (WSC * OSC))
                else:
                    nc.vector.tensor_scalar_mul(ys[:, nb, :], ops,
                                                1.0 / (WSC * OSC))
            elif ncp[0] % 2 == 0:
                nc.gpsimd.tensor_scalar_mul(ys[:, nb, :], ops,
                                            1.0 / (WSC * OSC))
            else:
                nc.vector.tensor_scalar_mul(ys[:, nb, :], ops,
                                            1.0 / (WSC * OSC))
            if split_dma:
                # end-critical tiles: one DMA per quarter, rotated across
                # queues so the final drain's DMA issues overlap
                eng = (nc.sync, nc.scalar, nc.sync, nc.gpsimd)[ncp[0] % 4]
                eng.dma_start(d_y[b, st, :, ns], ys[:, nb, :])
            elif nb == 3:
                pending_y.append((d_y[b, st], ys))
                flush_y(1)

        slab_cache = {}

        def emit_slab(h, j):
            # merged slab for all k-tile pairs of (h, j): [128, 2(j+1), 2, QB]
            gmax = 2 * (j + 1)
            qs = slice(j * QB, (j + 1) * QB)
            slab = slabs.tile([128, 8, 2, QB], FP8, tag="slab")
            nc.sync.dma_start(
                slab[:, 0:gmax],
                d_bias[h, 0:gmax * 256, qs].rearrange(
                    "(g r p) q -> p g r q", p=128, r=2))
            return slab

        # deferred work queue: (kind, closure) entries — v-transposes,
        # normalization tails and C-units — that fill PE gaps in later
        # m-loops. A tail's chain waits on the previous block's exp/U work,
        # so tails are only popped when `late` (the consumer has caught up);
        # C-units never jump ahead of their own block's tails.
        fill_q = []
        pending_fin = []   # deferred last-PV + tail of the previous m-loop
        popc = [0]         # pop_fill call counter (for entry locks)

        def flush_fin():
            while pending_fin:
                pending_fin.pop(0)()

        def pop_fill(k=1, late=True):
            # entries carry a lock (pop-call count before which they must not
            # run): a C-unit popped too soon stalls the PE on its tail's
            # oT writes.
            popc[0] += 1
            n = 0
            for _ in range(k):
                if not fill_q or (not draining[0]
                                  and fill_q[0][1] > popc[0]):
                    return n
                fill_q.pop(0)[2]()
                n += 1
            return n

        # ============ phase A: x^T panels -> q^T,k^T (RoPE), v ============
        qT = {}
        kT = {}
        vv = {}
        for b in range(B):
            for h in range(HPC):
                qT[b, h] = qkv.tile([128, S], BF16, tag=f"qT{b}{h}", name="qT")
                kT[b, h] = qkv.tile([128, S], BF16, tag=f"kT{b}{h}", name="kT")
                vv[b, h] = qkv.tile([128, ST, HD], BF16, tag=f"v{b}{h}", name="vv")

        def rope_emit(ps, dest, blk):
            # stage PSUM f32 -> SBUF bf16 once (ACT), then all-SBUF bf16
            # muls/adds run in the DVE's fast mode.
            # cs slot 0 = [cosT;cosT]/64, slot 1 = [-sinT;+sinT]/64
            prs = rope.tile([128, SB], BF16, tag="prs")
            nc.scalar.copy(prs, ps)
            t1 = rope.tile([128, SB], BF16, tag="t1")
            t2 = rope.tile([128, SB], BF16, tag="t2")
            nc.vector.tensor_mul(t1, prs, t_cs[:, 0, blk])
            nc.vector.tensor_mul(
                t2[0:64, :], prs[64:128, :], t_cs[0:64, 1, blk])
            nc.vector.tensor_mul(
                t2[64:128, :], prs[0:64, :], t_cs[64:128, 1, blk])
            nc.vector.tensor_add(dest[:, blk], t1, t2)

        def make_vtrans(b, h, sb_i, vts):
            # flip vT [hd, s] -> v [s, hd] with 4 PE transposes + one copy
            def vtrans():
                vtp = psum_l.tile([128, 4, 128], BF16, tag="l", name="vtp")
                for c4 in range(4):
                    cs4 = slice(c4 * 128, (c4 + 1) * 128)
                    nc.tensor.transpose(vtp[:, c4, :], vts[:, cs4], ident_bf)
                dstv = vv[b, h][:, sb_i * 4:(sb_i + 1) * 4, :]
                if ncp[0] % 2 == 0:
                    nc.scalar.copy(dstv, vtp)
                else:
                    nc.vector.tensor_copy(dstv, vtp)
                ncp[0] += 1
            return vtrans

        vq = []  # pending v-transposes (popped at the next block)

        for bi, (b, sb_i) in enumerate([(b, s) for b in range(B)
                                        for s in range(NSB)]):
            s0 = sb_i * SB
            blk = slice(s0, s0 + SB)
            if (b, sb_i) in panel_cache:
                phi, plo = panel_cache.pop((b, sb_i))
            elif bi == 0:
                # cold start: interleave the first panel's halves with the
                # weight tiles in consumption order
                blk0 = slice(0, SB)
                phi = panels.tile([128, GT, 2, SB], FP8, tag="phi", name="phi")
                plo = panels.tile([128, GT, 2, SB], FP8, tag="plo", name="plo")
                nc.sync.dma_start(phi[:, 0:GT // 2], d_xhi[0, :, 0:GT // 2, :, blk0])
                nc.sync.dma_start(phi[:, GT // 2:], d_xhi[0, :, GT // 2:, :, blk0])
                for nm in ("qhi", "khi"):
                    nc.sync.dma_start(t_w[nm][:, 0:GT // 2], d_w[nm][:, 0:GT // 2])
                nc.sync.dma_start(plo[:, 0:GT // 2], d_xlo[0, :, 0:GT // 2, :, blk0])
                nc.sync.dma_start(plo[:, GT // 2:], d_xlo[0, :, GT // 2:, :, blk0])
                for nm in ("qhi", "khi"):
                    nc.sync.dma_start(t_w[nm][:, GT // 2:], d_w[nm][:, GT // 2:])
                nc.sync.dma_start(t_cs[:, :, blk0], d_cs[:, :, blk0])
                for nm in ("qlo", "klo"):
                    nc.sync.dma_start(t_w[nm], d_w[nm])
            else:
                phi, plo = emit_panel(b, sb_i)
            # prefetch next block's panel while this one computes
            nxt = [(bb, ss) for bb in range(B) for ss in range(NSB)][bi + 1:bi + 2]
            if nxt and nxt[0] not in panel_cache:
                panel_cache[nxt[0]] = emit_panel(*nxt[0])
            if bi == 0:
                # v weights after the next panel: consumed later than it
                for nm in ("vhi", "vlo"):
                    nc.sync.dma_start(t_w[nm], d_w[nm])

            # q,k chains (both heads) first, then v: the first v chain
            # then starts after wv has streamed in.
            first = bi == 0
            if first:
                # cold start: hi*hi parts of all four q/k chains first
                # (split by panel half), so the PE works while the lo
                # tensors are still streaming in.
                chains = [(pre, h) for h in range(HPC) for pre in ("q", "k")]
                pss = {}
                for i, (pre, h) in enumerate(chains):
                    hc = slice(h * HD, (h + 1) * HD)
                    pool, tag = ((psum_mm, "mm") if i < 3 else
                                 (psum_c, "c"))
                    ps = pool.tile([128, SB], F32, tag=tag, name="pss")
                    pss[(pre, h)] = ps
                    dr_part(ps, [(t_w[pre + "hi"], phi, hc)], True, False,
                            gr=range(GT // 2))
                for pre, h in chains:
                    hc = slice(h * HD, (h + 1) * HD)
                    dr_part(pss[(pre, h)], [(t_w[pre + "hi"], phi, hc)],
                            False, False, gr=range(GT // 2, GT))
                for pre, h in chains:
                    hc = slice(h * HD, (h + 1) * HD)
                    dr_part(pss[(pre, h)], [(t_w[pre + "hi"], plo, hc)],
                            False, False)
                for pre, h in chains:
                    hc = slice(h * HD, (h + 1) * HD)
                    ps = pss[(pre, h)]
                    dr_part(ps, [(t_w[pre + "lo"], phi, hc)], False, True)
                    rope_emit(ps, (qT if pre == "q" else kT)[b, h], blk)
            else:
                for ci, (h, pre) in enumerate(
                        [(h, p) for h in range(HPC) for p in ("q", "k")]):
                    hc = slice(h * HD, (h + 1) * HD)
                    ps = psum_mm.tile([128, SB], F32, tag="mm")
                    dr_chain(ps, t_w[pre + "hi"], t_w[pre + "lo"],
                             phi, plo, hc)
                    rope_emit(ps, (qT if pre == "q" else kT)[b, h], blk)
                    if ci == 0 and vq:
                        # previous block's v-transposes: the staging copy
                        # they wait on has finished during this chain
                        for f in vq:
                            f()
                        vq.clear()
            if first and vq:
                pass
            # v chains, transposed like q/k, then staged to SBUF bf16
            for h in range(HPC):
                hc = slice(h * HD, (h + 1) * HD)
                ps = psum_mm.tile([128, SB], F32, tag="mm")
                dr_chain(ps, t_w["vhi"], t_w["vlo"], phi, plo, hc)
                vts = vtsp.tile([128, SB], BF16, tag="vts")
                nc.scalar.activation(vts, ps, CPY, 0.0, 1.0 / WSC)
                vq.append(make_vtrans(b, h, sb_i, vts))
            if bi == 0:
                # on ACT, not Pool: any DMA in Pool's stream lands before
                # make_identity in the scheduled order and the warm-up
                # fillers' semaphore would wait on its completion.
                nc.sync.dma_start(t_logw, d_logw)
                nc.sync.dma_start(id8, d_id8)
            if bi == 6:
                # prefetch phase B's first slabs
                slab_cache[(0, 0)] = emit_slab(0, 0)
                slab_cache[(1, 0)] = emit_slab(1, 0)
            if bi == 7:
                nc.sync.dma_start(t_woh, d_woh)
                nc.sync.dma_start(t_wol, d_wol)
        # last block's v-transposes run inside the first m-loop (fill_q)
        for f in vq:
            fill_q.append(("v", 0, f))
        vq = []

        # attn output o^T, fp8 hi/lo, heads packed on the DoubleRow pair
        # axis for the output projection: [128(hd), HPC, S]
        oT = {}
        for b in range(B):
            oT[b, "h"] = qkv.tile([128, HPC, S], FP8, tag=f"oh{b}", name="oh")
            oT[b, "l"] = qkv.tile([128, HPC, S], FP8, tag=f"ol{b}", name="ol")

        # ====== phases B+C software-pipelined over q-blocks, batch-inner ======
        for j in range(NQB):
            qs = slice(j * QB, (j + 1) * QB)
            nk = 4 * (j + 1)          # causal: k-tiles 0..nk-1
            for h in range(HPC):
                slab = slab_cache.pop((h, j), None)
                if slab is None:
                    slab = emit_slab(h, j)
                # prefetch the next (h, j) slab
                if h + 1 < HPC:
                    if (h + 1, j) not in slab_cache:
                        slab_cache[(h + 1, j)] = emit_slab(h + 1, j)
                elif j + 1 < NQB and (0, j + 1) not in slab_cache:
                    slab_cache[(0, j + 1)] = emit_slab(0, j + 1)
                for b in range(B):
                    pv = psum_pv.tile([128, QB], F32, tag="pv", name="pv")
                    U = upool.tile([128, QB], BF16, tag="U", name="U")
                    offs = [max(0, (m - 4 * j) * 128) for m in range(nk)]

                    def emit_pvu(m, ex, pv=pv, U=U, b=b, h=h, nk=nk,
                                 offs=offs):
                        off = offs[m]
                        nc.tensor.matmul(
                            pv[:, off:], vv[b, h][:, m, :], ex[:, off:],
                            start=(m == 0), stop=(m == nk - 1),
                            skip_group_check=True)
                        if m == 0:
                            nc.vector.tensor_copy(U, ex)
                        else:
                            nc.vector.tensor_add(
                                U[:, off:], U[:, off:], ex[:, off:])

                    # software-skewed m loop: scores/bias/exp of m, then the
                    # PV+U of m-1 (covering the exp latency with fill pops).
                    # The last PV + normalization tail are deferred into the
                    # NEXT m-loop (emitted behind its first QK+bias), so the
                    # final exp's latency never stalls the PE.
                    prev = None
                    sc_next = None
                    if not fill_q and not pending_fin:
                        sc_next = psum_mm.tile([128, QB], F32, tag="mm",
                                               name="sc_next")
                        for _ in range(3 if j == 0 and h == 0 else 2):
                            nc.tensor.matmul(
                                sc_next, ident_bf, qT[b, h][:, qs],
                                start=True, stop=True, skip_group_check=True)
                    for m in range(nk):
                        ml = m % 2
                        off = offs[m]
                        qso = slice(j * QB + off, (j + 1) * QB)
                        if sc_next is not None:
                            sc = sc_next
                            sc_next = None
                        else:
                            sc = psum_mm.tile([128, QB], F32, tag="mm")
                        nc.tensor.matmul(
                            sc[:, off:], kT[b, h][:, m * 128:(m + 1) * 128],
                            qT[b, h][:, qso],
                            start=True, stop=False)
                        nc.tensor.matmul(
                            sc[:, off:], id8[:, ml:ml + 2, :],
                            slab[:, m // 2, :, off:],
                            start=False, stop=True, perf_mode=DR,
                            skip_group_check=True)
                        ex = expp.tile([128, QB], BF16, tag="ex")
                        nc.scalar.activation(ex[:, off:], sc[:, off:], EXP,
                                             bias=t_logw[:, b, m:m + 1])
                        if m == 0:
                            flush_fin()
                        if prev is not None:
                            if b == B - 1 and j == NQB - 1 and h == HPC - 1:
                                # keep a few C-units back: they fill the PE
                                # during the final tail's serial chain
                                k = 1 if m % 2 else 0
                            else:
                                k = 2 if len(fill_q) > 8 else 1
                            pop_fill(k, late=(m >= 3))
                            emit_pvu(*prev)
                        prev = (m, ex)

                    last = (b == B - 1 and j == NQB - 1 and h == HPC - 1)

                    def make_fin(prev=prev, pv=pv, U=U, b=b, h=h, qs=qs,
                                 j=j, last=last, emit_pvu=emit_pvu):
                        def fin():
                            emit_pvu(*prev)
                            # tail: no PE work (partition_all_reduce replaces
                            # the ones-matmul); runs on Pool/DVE/ACT behind
                            # the current m-loop. Row sums (all partitions),
                            # reciprocal, normalize, fp8 hi/lo of o^T * 64.
                            if last:
                                # end-critical: chunk the chain so the first
                                # drain C-units (one 128-col oT slice each)
                                # start before the later chunks normalize
                                chunks = [slice(c * 128, (c + 1) * 128)
                                          for c in range(QB // 128)]
                            else:
                                chunks = [slice(0, QB)]
                            L = normp.tile([128, QB], F32, tag="L")
                            rb = normp.tile([128, QB], F32, tag="rb")
                            t = normp.tile([128, QB], BF16, tag="t")
                            for cs4 in chunks:
                                qc = slice(qs.start + cs4.start,
                                           qs.start + cs4.stop)
                                nc.gpsimd.partition_all_reduce(
                                    L[:, cs4], U[:, cs4], 128, RADD)
                                nc.vector.reciprocal(rb[:, cs4], L[:, cs4])
                                nc.vector.tensor_mul(
                                    t[:, cs4], pv[:, cs4], rb[:, cs4])
                                nc.scalar.activation(
                                    oT[b, "h"][:, h, qc], t[:, cs4], CPY,
                                    0.0, OSC)
                                nc.gpsimd.scalar_tensor_tensor(
                                    oT[b, "l"][:, h, qc], t[:, cs4], OSC,
                                    oT[b, "h"][:, h, qc], MUL, SUB)
                            if h == HPC - 1:
                                # both heads of (b, j) normalized: its
                                # C-units can pop once the tail chain has
                                # had ~10 pop-calls of headroom
                                fill_q.extend(
                                    ("c", popc[0] + 10,
                                     lambda st=st, nb=nb, b=b:
                                     emit_c_unit(
                                         b, st, nb, oT[b, "h"], oT[b, "l"],
                                         split_dma=(j == NQB - 1)))
                                    for st in range(4 * j, 4 * j + 4)
                                    for nb in range(4))
                        return fin

                    pending_fin.append(make_fin())

        flush_fin()
        draining[0] = True
        while fill_q:
            fill_q.pop(0)[2]()
        flush_y(len(pending_y))

    nc.compile()
    return nc


def _host_prep(x, Wq, Wk, Wv, Wo, policy_mask, memory_weights):
    """Build the per-core input maps."""
    bf = ml_dtypes.bfloat16
    f8 = ml_dtypes.float8_e4m3

    def hilo(a):
        hi = a.astype(f8)
        lo = (a - hi.astype(np.float32)).astype(f8)
        return hi, lo

    def hilo_tiles(a):
        # [D, C] (or [D, S]) -> hi/lo fp8 in [128, GT, 2, C] DoubleRow layout
        hi, lo = hilo(a)
        def tl(t):
            return np.ascontiguousarray(
                t.reshape(GT, 2, 128, -1).transpose(2, 0, 1, 3))
        return tl(hi), tl(lo)

    xhi = np.empty((B, 128, GT, 2, S), f8)
    xlo = np.empty((B, 128, GT, 2, S), f8)
    for b in range(B):
        xt = np.ascontiguousarray(np.asarray(x[b], np.float32).T)  # [D, S]
        xhi[b], xlo[b] = hilo_tiles(xt)

    # RoPE tables (carry the 1/WSC weight descale):
    inv_freq = (1.0 / (ROPE_BASE ** (np.arange(0, HD, 2, dtype=np.float32) / HD)))
    t = np.arange(S, dtype=np.float32)
    freqs = np.outer(t, inv_freq).astype(np.float32)      # [S, 64]
    cosT = np.cos(freqs).T.astype(np.float32) / WSC       # [64, S]
    sinT = np.sin(freqs).T.astype(np.float32) / WSC
    cs = np.empty((128, 2, S), np.float32)
    cs[0:64, 0] = cosT
    cs[64:128, 0] = cosT
    cs[0:64, 1] = -sinT
    cs[64:128, 1] = sinT
    cs = cs.astype(bf)

    # memory multiplier w = 1 + GS*mw + 1e-8  (exp(log1p(z)) = 1+z);
    # exact f32, applied as the exp's per-partition bias: [128, B, ST]
    mw = memory_weights.reshape(B, S).astype(np.float64)
    logw = np.log(1.0 + GS * mw + 1e-8).astype(np.float32)  # [B, S]
    logw_t = np.ascontiguousarray(
        logw.reshape(B, ST, 128).transpose(2, 0, 1))        # [128, B, ST]

    # transposed, causal-masked, pre-scaled policy bias per head
    # (batch-independent: log w rides the exp bias operand instead)
    maskT = np.tril(np.full((S, S), MASK_NEG, dtype=np.float32), -1)
    pol = np.asarray(policy_mask, dtype=np.float32)[0]    # [H, S, S]

    id8h = np.zeros((128, 3, 128), np.float32)
    id8h[:, 0, :] = np.eye(128, dtype=np.float32)
    id8h[:, 2, :] = np.eye(128, dtype=np.float32)
    id8h = id8h.astype(f8)

    in_maps = []
    for c in range(NCORES):
        cols = slice(c * HPC * HD, (c + 1) * HPC * HD)
        bias_c = np.empty((HPC, S, S), dtype=f8)
        for hl in range(HPC):
            hg = c * HPC + hl
            bias_c[hl] = (GS * pol[hg].T + maskT).astype(f8)
        wo_c = np.ascontiguousarray(
            np.asarray(Wo, np.float32)[cols, :]
            .reshape(HPC, 128, D).transpose(1, 0, 2)) * np.float32(WSC)
        woh, wol = hilo(wo_c)
        m = {"xhi": xhi, "xlo": xlo, "woh": woh, "wol": wol,
             "biasT": bias_c, "cs": cs, "id8": id8h, "logw": logw_t}
        for nm, w, s in (("q", Wq, WSC), ("k", Wk, WSC * SCALE), ("v", Wv, WSC)):
            hi, lo = hilo_tiles(np.asarray(w, np.float32)[:, cols] * np.float32(s))
            m[f"w{nm}hi"] = hi
            m[f"w{nm}lo"] = lo
        in_maps.append(m)
    return in_maps


def kernel(x, Wq, Wk, Wv, Wo, bo, policy_mask, memory_weights):
    x = np.asarray(x, dtype=np.float32)
    Wq = np.asarray(Wq, dtype=np.float32)
    Wk = np.asarray(Wk, dtype=np.float32)
    Wv = np.asarray(Wv, dtype=np.float32)
    Wo = np.asarray(Wo, dtype=np.float32)
    bo = np.asarray(bo, dtype=np.float32)

    if "nc" not in _CACHE:
        _CACHE["nc"] = build_nc()
    nc = _CACHE["nc"]

    in_maps = _host_prep(x, Wq, Wk, Wv, Wo, policy_mask, memory_weights)
    res = run_bass_kernel_spmd(nc, in_maps, core_ids=list(range(NCORES)))

    acc = np.zeros((B, S, D), dtype=np.float64)
    for c in range(NCORES):
        acc += res.results[c]["y"].astype(np.float64).reshape(B, S, D)
    return (acc + bo.astype(np.float64)).astype(np.float32)
